# revision 18
# baseline (speedup 1.0000x reference)
"""Trainium2 Bass kernel for BSplineLayer: y = BSpline(knots, coeffs, k=3)((x - min(x)) / (max(x) - min(x) + 1e-8)).

The reference clips the de Boor interval index to [k, n-1] = [3, 3], so the
layer is a single cubic P_unit(z) evaluated at z = s*x + b where s, b come
from the global min/max.  Host-side we factor the cubic exactly (every real
cubic has a real root r):

    P_unit(z) = a3 * (z - r) * ((z + U/2)^2 + D)

so the device evaluates, per element, just three streaming passes:

    sqp = Square(s*x + (b + U/2))        # ACT, scale/bias are runtime APs
    t1  = K1*x + K2                      # a3*(z - r); DVE TS @2x or ACT affine
    y   = (D + sqp) * t1                 # DVE scalar_tensor_tensor

K1 = a3*s, K2 = a3*(b - r); r, U/2, D, a3 are compile-time immediates
(program cache is keyed on knots/coeffs bytes).  The t1 pass alternates
ACT/DVE so both engines sit at ~44us, under the ~47us of store DMA.

Phase 1 keeps the GPSIMD Q7 cores idle on purpose: the ncfw warm-up
collective's lazy comm bring-up runs on them, and any Q7 work in this window
delays the real AllReduce by tens of us (measured).  DVE alone scans for
min+max (2 x 4.33us per [128,4096] tile, ~71us total) while tiles stream in;
x stays SBUF-resident (16 MiB) so HBM traffic is one read + one write.
"""

import sys

sys.path.insert(0, "/opt/trn_rl_repo")

import numpy as np

N_CORES = 8
ROWS, COLS = 8192, 4096
R_CORE = ROWS // N_CORES          # 1024 rows per core
P = 128                           # SBUF partitions
N_TILES = R_CORE // P             # 8 tiles of [128, 4096] per core
CHUNK = 4096                      # phase-2 free-dim chunk (one tile)
DEGREE = 3

_CACHE = {}


def _expand_cubic(knots: np.ndarray, coeffs: np.ndarray) -> np.ndarray:
    """Expand de Boor at interval m=3 into monomial coeffs [a0, a1, a2, a3] (float64)."""
    t = np.asarray(knots, dtype=np.float64)
    c = np.asarray(coeffs, dtype=np.float64)
    k = DEGREE
    m = k  # reference clips searchsorted result to [k, n-1] with n-1 == k
    pm = np.polynomial.polynomial
    d = [np.array([c[m - k + j]], dtype=np.float64) for j in range(k + 1)]
    for r in range(1, k + 1):
        for j in range(k, r - 1, -1):
            tl = t[m - k + j]
            tr = t[m + j + 1 - r]
            inv = 1.0 / (tr - tl)
            alpha = np.array([-tl * inv, inv])
            one_m = np.array([1.0 + tl * inv, -inv])
            d[j] = pm.polyadd(pm.polymul(one_m, d[j - 1]), pm.polymul(alpha, d[j]))
    a = np.zeros(4, dtype=np.float64)
    a[: len(d[k])] = d[k]
    return a


def _factor_cubic(a: np.ndarray):
    """P(z) = a3*(z - r)*((z + U/2)^2 + D) with real r, U, D (float64)."""
    a3 = a[3] if abs(a[3]) > 1e-30 else 1e-30
    roots = np.roots([a3, a[2], a[1], a[0]])
    # pick the real root (guaranteed >= 1); among real roots take the one
    # farthest from the z in [0, 1] working domain for conditioning
    real = [z for z in roots if abs(z.imag) <= 1e-9 * max(1.0, abs(z.real))]
    if not real:  # numerically-forced fallback: most-real root
        real = [min(roots, key=lambda z: abs(z.imag))]
    r = max(real, key=lambda z: abs(z.real - 0.5)).real
    rem = sorted(roots, key=lambda z: abs(z.real - r) + abs(z.imag))[1:]
    U = float(-(rem[0] + rem[1]).real)
    V = float((rem[0] * rem[1]).real)
    D = V - 0.25 * U * U
    return float(r), float(U), float(D), float(a3)


def _build_program(r: float, U: float, D: float, a3: float):
    import concourse.bass as bass
    import concourse.tile as tile
    from concourse import bacc, bass_isa, mybir

    dt = mybir.dt.float32
    OP = mybir.AluOpType
    AX = mybir.AxisListType
    AF = mybir.ActivationFunctionType

    nc = bacc.Bacc("TRN2", target_bir_lowering=False, debug=False, num_devices=N_CORES)
    x_ext = nc.declare_dram_parameter("x", [R_CORE, COLS], dt, isOutput=False)
    y_ext = nc.declare_dram_parameter("y", [R_CORE, COLS], dt, isOutput=True)

    with tile.TileContext(nc) as tc:
        with (
            tc.tile_pool(name="xp", bufs=1) as xp,
            tc.tile_pool(name="sqp", bufs=2) as sqp_pool,
            tc.tile_pool(name="wp", bufs=2) as wp,
            tc.tile_pool(name="small", bufs=1) as small,
            tc.tile_pool(name="dram", bufs=1, space="DRAM") as dram,
        ):
            # Warm the collective path (ncfw queue/ring setup + core-skew
            # sync) concurrently with the phase-1 loads so the real AllReduce
            # is cheap.  Gathers an uninitialized DRAM word on purpose: zero
            # dependencies means the gpsimd stream enqueues it immediately.
            warm_in = dram.tile([1, 2], dt)
            warm_out = dram.tile([1, 2], dt)
            nc.gpsimd.collective_compute(
                "AllReduce", OP.max,
                replica_groups=[list(range(N_CORES))],
                ins=[warm_in[:].opt()], outs=[warm_out[:].opt()],
            )

            # ACT table warm-up: the first activation triggers the table-set
            # DMA (~2.7us); run it on a [P,1] scratch during phase 1, off the
            # post-collective critical path.
            actwarm = small.tile([P, 2], dt)
            nc.vector.memset(actwarm[:, 0:1], 0.0)
            nc.scalar.activation(actwarm[:, 1:2], actwarm[:, 0:1], AF.Square,
                                 bias=0.0, scale=1.0)
            nc.scalar.activation(actwarm[:, 1:2], actwarm[:, 0:1], AF.Identity,
                                 bias=0.0, scale=1.0)

            # ---------------- phase 1: load + local min/max ----------------
            # DVE-only reduces (the Q7s must stay idle -- see module doc).
            # Tile 0 loads in quarters and tile 1 in halves so the first
            # reduce starts as soon as the first 512 KiB lands; DVE stays
            # saturated to the end, so an earlier start shifts the whole
            # pipeline left.
            xts = []
            for t in range(N_TILES):
                xt = xp.tile([P, COLS], dt, tag=f"x{t}")
                xts.append(xt)

            Q, H = COLS // 4, COLS // 2
            pieces = [(0, q * Q, (q + 1) * Q) for q in range(4)]
            pieces += [(1, 0, H), (1, H, COLS)]
            pieces += [(t, 0, COLS) for t in range(2, N_TILES)]
            for t, lo, hi in pieces:
                nc.sync.dma_start(out=xts[t][:, lo:hi],
                                  in_=x_ext[t * P:(t + 1) * P, lo:hi])

            NP = len(pieces)
            rmin = small.tile([P, NP], dt)
            rmax = small.tile([P, NP], dt)
            for i, (t, lo, hi) in enumerate(pieces):
                a = xts[t][:, lo:hi]
                nc.vector.tensor_reduce(rmax[:, i:i + 1], a,
                                        axis=AX.X, op=OP.max)
                nc.vector.tensor_reduce(rmin[:, i:i + 1], a,
                                        axis=AX.X, op=OP.min)

            pk = small.tile([P, 2], dt)
            nc.vector.tensor_reduce(pk[:, 0:1], rmax[:], axis=AX.X, op=OP.max)
            rmn = small.tile([P, 1], dt)
            nc.vector.tensor_reduce(rmn[:], rmin[:], axis=AX.X, op=OP.min)
            nc.vector.tensor_scalar_mul(pk[:, 1:2], rmn[:], -1.0)

            # cross-partition: every partition gets (local_max, -local_min)
            par = small.tile([P, 2], dt)
            nc.gpsimd.partition_all_reduce(par[:], pk[:], channels=P,
                                           reduce_op=bass_isa.ReduceOp.max)

            # cross-core: AllReduce(max) of the pair
            cc_in = dram.tile([1, 2], dt)
            cc_out = dram.tile([1, 2], dt)
            nc.sync.dma_start(out=cc_in[:], in_=par[0:1, 0:2])
            nc.gpsimd.collective_compute(
                "AllReduce", OP.max,
                replica_groups=[list(range(N_CORES))],
                ins=[cc_in[:].opt()], outs=[cc_out[:].opt()],
            )
            GG = small.tile([P, 2], dt)
            nc.sync.dma_start(out=GG[:], in_=cc_out[:].partition_broadcast(P))

            # ------- device scalars: s, b and phase-2 coefficients -------
            # s = 1/(gmax + gnm + eps); b = gnm*s      (gnm = -gmin)
            # bias2 = b + U/2; K1 = a3*s; K2 = a3*(b - r)
            cf = small.tile([P, 6], dt)
            dd, s_, b_, bias2, K1, K2 = (cf[:, i:i + 1] for i in range(6))
            nc.vector.scalar_tensor_tensor(dd, GG[:, 0:1], 1e-8, GG[:, 1:2],
                                           op0=OP.add, op1=OP.add)
            nc.vector.reciprocal(s_, dd)
            nc.vector.tensor_tensor(b_, GG[:, 1:2], s_, op=OP.mult)
            nc.vector.tensor_scalar_add(bias2, b_, 0.5 * U)
            nc.vector.tensor_scalar_mul(K1, s_, a3)
            tb_ = small.tile([P, 1], dt)
            nc.vector.tensor_scalar_add(tb_, b_, -r)
            nc.vector.tensor_scalar_mul(K2, tb_, a3)

            # ACT-owned copy of (s, bias2, K1): phase-2 ACT ops then wait on
            # at most one foreign semaphore.
            acoef = small.tile([P, 4], dt)
            nc.scalar.copy(acoef[:, 0:2], cf[:, 1:3])   # s_, b_ (b_ unused)
            nc.scalar.copy(acoef[:, 2:4], cf[:, 3:5])   # bias2, K1
            s_a = acoef[:, 0:1]
            bias2_a = acoef[:, 2:3]
            K1_a = acoef[:, 3:4]
            K2_d = K2  # DVE-side TS reads cf directly

            # ---------------- phase 2: evaluate + store ----------------
            # One chunk per tile ([128, 4096]); the last tile is split into
            # halves so the compute+store tail after the final STT is ~3us
            # shorter.  Per chunk: ACT Square (~3.7us), t1 affine alternating
            # ACT/DVE (balances both engines at ~41-44us), the combining STT
            # on DVE (4.33us) in place over x, then the store.  Store DMA
            # (~47us total) is the pacer.
            chunks = [(t, 0, COLS) for t in range(N_TILES - 1)]
            chunks += [(N_TILES - 1, 0, H), (N_TILES - 1, H, COLS)]
            for ci, (t, lo, hi) in enumerate(chunks):
                xc = xts[t][:, lo:hi]
                fd = hi - lo
                sq = sqp_pool.tile([P, CHUNK], dt, tag="sq")
                nc.scalar.activation(sq[:, :fd], xc, AF.Square,
                                     bias=bias2_a, scale=s_a)
                t1 = wp.tile([P, CHUNK], dt, tag="t1")
                if ci % 2 == 1:
                    nc.scalar.activation(t1[:, :fd], xc, AF.Identity,
                                         bias=K2_d, scale=K1_a)
                else:
                    nc.vector.tensor_scalar(t1[:, :fd], xc, K1_a, K2_d,
                                            op0=OP.mult, op1=OP.add)
                nc.vector.scalar_tensor_tensor(xc, sq[:, :fd], float(D),
                                               t1[:, :fd],
                                               op0=OP.add, op1=OP.mult)
                nc.sync.dma_start(out=y_ext[t * P:(t + 1) * P, lo:hi], in_=xc)

    nc.compile()
    return nc


def kernel(x: np.ndarray, knots: np.ndarray, coeffs: np.ndarray) -> np.ndarray:
    from concourse.bass_utils import run_bass_kernel_spmd

    x = np.ascontiguousarray(np.asarray(x, dtype=np.float32))
    assert x.shape == (ROWS, COLS), x.shape

    a = _expand_cubic(knots, coeffs)
    r, U, D, a3 = _factor_cubic(a)

    key = (np.asarray(knots, np.float32).tobytes(),
           np.asarray(coeffs, np.float32).tobytes())
    if _CACHE.get("key") != key:
        _CACHE["nc"] = _build_program(r, U, D, a3)
        _CACHE["key"] = key
    nc = _CACHE["nc"]

    shards = [x[i * R_CORE:(i + 1) * R_CORE] for i in range(N_CORES)]
    in_maps = [{"x": s} for s in shards]

    import os
    trace = bool(int(os.environ.get("KERNEL_TRACE", "0")))
    res = run_bass_kernel_spmd(nc, in_maps, core_ids=list(range(N_CORES)),
                               trace=trace)
    if trace and res.exec_time_ns is not None:
        print(f"HW exec time: {res.exec_time_ns} ns")
        _CACHE["last_exec_time_ns"] = res.exec_time_ns
        _CACHE["last_trace"] = res.instructions_and_trace

    out = np.empty((ROWS, COLS), dtype=np.float32)
    for i in range(N_CORES):
        out[i * R_CORE:(i + 1) * R_CORE] = res.results[i]["y"]
    return out


# revision 22
# speedup vs baseline: 1.0609x; 1.0609x over previous
"""Trainium2 Bass kernel for BSplineLayer: y = BSpline(knots, coeffs, k=3)((x - min(x)) / (max(x) - min(x) + 1e-8)).

The reference clips the de Boor interval index to [k, n-1] = [3, 3], so the
layer is a single cubic P_unit(z) evaluated at z = s*x + b where s, b come
from the global min/max.  Host-side we factor the cubic exactly (every real
cubic has a real root r):

    P_unit(z) = a3 * (z - r) * ((z + U/2)^2 + D)

so the device evaluates, per element, just three streaming passes:

    sqp = Square(s*x + (b + U/2))        # ACT, scale/bias are runtime APs
    t1  = K1*x + K2                      # a3*(z - r); DVE TS @2x or ACT affine
    y   = (D + sqp) * t1                 # DVE scalar_tensor_tensor

K1 = a3*s, K2 = a3*(b - r); r, U/2, D, a3 are compile-time immediates
(program cache is keyed on knots/coeffs bytes).  The t1 pass alternates
ACT/DVE so both engines sit at ~44us, under the ~47us of store DMA.

Phase 1 keeps the GPSIMD Q7 cores idle on purpose: the ncfw warm-up
collective's lazy comm bring-up runs on them, and any Q7 work in this window
delays the real AllReduce by tens of us (measured).  DVE alone scans for
min+max (2 x 4.33us per [128,4096] tile, ~71us total) while tiles stream in;
x stays SBUF-resident (16 MiB) so HBM traffic is one read + one write.
"""

import sys

sys.path.insert(0, "/opt/trn_rl_repo")

import numpy as np

N_CORES = 8
ROWS, COLS = 8192, 4096
R_CORE = ROWS // N_CORES          # 1024 rows per core
P = 128                           # SBUF partitions
N_TILES = R_CORE // P             # 8 tiles of [128, 4096] per core
CHUNK = 4096                      # phase-2 free-dim chunk (one tile)
DEGREE = 3

_CACHE = {}


def _expand_cubic(knots: np.ndarray, coeffs: np.ndarray) -> np.ndarray:
    """Expand de Boor at interval m=3 into monomial coeffs [a0, a1, a2, a3] (float64)."""
    t = np.asarray(knots, dtype=np.float64)
    c = np.asarray(coeffs, dtype=np.float64)
    k = DEGREE
    m = k  # reference clips searchsorted result to [k, n-1] with n-1 == k
    pm = np.polynomial.polynomial
    d = [np.array([c[m - k + j]], dtype=np.float64) for j in range(k + 1)]
    for r in range(1, k + 1):
        for j in range(k, r - 1, -1):
            tl = t[m - k + j]
            tr = t[m + j + 1 - r]
            inv = 1.0 / (tr - tl)
            alpha = np.array([-tl * inv, inv])
            one_m = np.array([1.0 + tl * inv, -inv])
            d[j] = pm.polyadd(pm.polymul(one_m, d[j - 1]), pm.polymul(alpha, d[j]))
    a = np.zeros(4, dtype=np.float64)
    a[: len(d[k])] = d[k]
    return a


def _factor_cubic(a: np.ndarray):
    """P(z) = a3*(z - r)*((z + U/2)^2 + D) with real r, U, D (float64)."""
    a3 = a[3] if abs(a[3]) > 1e-30 else 1e-30
    roots = np.roots([a3, a[2], a[1], a[0]])
    # pick the real root (guaranteed >= 1); among real roots take the one
    # farthest from the z in [0, 1] working domain for conditioning
    real = [z for z in roots if abs(z.imag) <= 1e-9 * max(1.0, abs(z.real))]
    if not real:  # numerically-forced fallback: most-real root
        real = [min(roots, key=lambda z: abs(z.imag))]
    r = max(real, key=lambda z: abs(z.real - 0.5)).real
    rem = sorted(roots, key=lambda z: abs(z.real - r) + abs(z.imag))[1:]
    U = float(-(rem[0] + rem[1]).real)
    V = float((rem[0] * rem[1]).real)
    D = V - 0.25 * U * U
    return float(r), float(U), float(D), float(a3)


def _build_program(r: float, U: float, D: float, a3: float):
    import concourse.bass as bass
    import concourse.tile as tile
    from concourse import bacc, bass_isa, mybir

    dt = mybir.dt.float32
    OP = mybir.AluOpType
    AX = mybir.AxisListType
    AF = mybir.ActivationFunctionType

    nc = bacc.Bacc("TRN2", target_bir_lowering=False, debug=False, num_devices=N_CORES)
    x_ext = nc.declare_dram_parameter("x", [R_CORE, COLS], dt, isOutput=False)
    y_ext = nc.declare_dram_parameter("y", [R_CORE, COLS], dt, isOutput=True)

    with tile.TileContext(nc) as tc:
        with (
            tc.tile_pool(name="xp", bufs=1) as xp,
            tc.tile_pool(name="sqp", bufs=2) as sqp_pool,
            tc.tile_pool(name="wp", bufs=2) as wp,
            tc.tile_pool(name="small", bufs=1) as small,
            tc.tile_pool(name="dram", bufs=1, space="DRAM") as dram,
        ):
            # Warm the collective path (ncfw queue/ring setup + core-skew
            # sync) concurrently with the phase-1 loads so the real AllReduce
            # is cheap.  Gathers an uninitialized DRAM word on purpose: zero
            # dependencies means the gpsimd stream enqueues it immediately.
            warm_in = dram.tile([1, 2], dt)
            warm_out = dram.tile([1, 2], dt)
            nc.gpsimd.collective_compute(
                "AllReduce", OP.max,
                replica_groups=[list(range(N_CORES))],
                ins=[warm_in[:].opt()], outs=[warm_out[:].opt()],
            )

            # ACT table warm-up: the first activation triggers the table-set
            # DMA (~2.7us); run it on a [P,1] scratch during phase 1, off the
            # post-collective critical path.
            actwarm = small.tile([P, 2], dt)
            nc.vector.memset(actwarm[:, 0:1], 0.0)
            nc.scalar.activation(actwarm[:, 1:2], actwarm[:, 0:1], AF.Square,
                                 bias=0.0, scale=1.0)
            nc.scalar.activation(actwarm[:, 1:2], actwarm[:, 0:1], AF.Identity,
                                 bias=0.0, scale=1.0)

            # ---------------- phase 1: load + local min/max ----------------
            # DVE-only reduces (the Q7s must stay idle -- see module doc).
            # Tile 0 loads in quarters and tile 1 in halves so the first
            # reduce starts as soon as the first 512 KiB lands; DVE stays
            # saturated to the end, so an earlier start shifts the whole
            # pipeline left.
            xts = []
            for t in range(N_TILES):
                xt = xp.tile([P, COLS], dt, tag=f"x{t}")
                xts.append(xt)

            Q, H = COLS // 4, COLS // 2
            pieces = [(0, q * Q, (q + 1) * Q) for q in range(4)]
            pieces += [(1, 0, H), (1, H, COLS)]
            pieces += [(t, 0, COLS) for t in range(2, N_TILES)]
            for t, lo, hi in pieces:
                nc.sync.dma_start(out=xts[t][:, lo:hi],
                                  in_=x_ext[t * P:(t + 1) * P, lo:hi])

            NP = len(pieces)
            rmin = small.tile([P, NP], dt)
            rmax = small.tile([P, NP], dt)
            for i, (t, lo, hi) in enumerate(pieces):
                a = xts[t][:, lo:hi]
                nc.vector.tensor_reduce(rmax[:, i:i + 1], a,
                                        axis=AX.X, op=OP.max)
                nc.vector.tensor_reduce(rmin[:, i:i + 1], a,
                                        axis=AX.X, op=OP.min)

            pk = small.tile([P, 2], dt)
            nc.vector.tensor_reduce(pk[:, 0:1], rmax[:], axis=AX.X, op=OP.max)
            rmn = small.tile([P, 1], dt)
            nc.vector.tensor_reduce(rmn[:], rmin[:], axis=AX.X, op=OP.min)
            nc.vector.tensor_scalar_mul(pk[:, 1:2], rmn[:], -1.0)

            # cross-partition: every partition gets (local_max, -local_min)
            par = small.tile([P, 2], dt)
            nc.gpsimd.partition_all_reduce(par[:], pk[:], channels=P,
                                           reduce_op=bass_isa.ReduceOp.max)

            # cross-core: AllReduce(max) of the pair
            cc_in = dram.tile([1, 2], dt)
            cc_out = dram.tile([1, 2], dt)
            nc.sync.dma_start(out=cc_in[:], in_=par[0:1, 0:2])
            nc.gpsimd.collective_compute(
                "AllReduce", OP.max,
                replica_groups=[list(range(N_CORES))],
                ins=[cc_in[:].opt()], outs=[cc_out[:].opt()],
            )
            GG = small.tile([P, 2], dt)
            nc.sync.dma_start(out=GG[:], in_=cc_out[:].partition_broadcast(P))

            # ------- device scalars: s, b and phase-2 coefficients -------
            # s = 1/(gmax + gnm + eps); b = gnm*s      (gnm = -gmin)
            # bias2 = b + U/2; K1 = a3*s; K2 = a3*(b - r)
            cf = small.tile([P, 6], dt)
            dd, s_, b_, bias2, K1, K2 = (cf[:, i:i + 1] for i in range(6))
            nc.vector.scalar_tensor_tensor(dd, GG[:, 0:1], 1e-8, GG[:, 1:2],
                                           op0=OP.add, op1=OP.add)
            nc.vector.reciprocal(s_, dd)
            nc.vector.tensor_tensor(b_, GG[:, 1:2], s_, op=OP.mult)
            nc.vector.tensor_scalar_add(bias2, b_, 0.5 * U)
            nc.vector.tensor_scalar_mul(K1, s_, a3)
            tb_ = small.tile([P, 1], dt)
            nc.vector.tensor_scalar_add(tb_, b_, -r)
            nc.vector.tensor_scalar_mul(K2, tb_, a3)

            # ACT-owned copy of (s, bias2, K1): phase-2 ACT ops then wait on
            # at most one foreign semaphore.
            acoef = small.tile([P, 4], dt)
            nc.scalar.copy(acoef[:, 0:2], cf[:, 1:3])   # s_, b_ (b_ unused)
            nc.scalar.copy(acoef[:, 2:4], cf[:, 3:5])   # bias2, K1
            s_a = acoef[:, 0:1]
            bias2_a = acoef[:, 2:3]
            K1_a = acoef[:, 3:4]
            K2_d = K2  # DVE-side TS reads cf directly

            # ---------------- phase 2: evaluate + store ----------------
            # One chunk per tile ([128, 4096]); the last tile is split into
            # halves so the compute+store tail after the final STT is ~3us
            # shorter.  Per chunk: ACT Square (~3.7us), t1 affine alternating
            # ACT/DVE (balances both engines at ~41-44us), the combining STT
            # on DVE (4.33us) in place over x, then the store.  Store DMA
            # (~47us total) is the pacer.
            chunks = [(t, 0, COLS) for t in range(N_TILES - 1)]
            chunks += [(N_TILES - 1, 0, H), (N_TILES - 1, H, COLS)]
            for ci, (t, lo, hi) in enumerate(chunks):
                xc = xts[t][:, lo:hi]
                fd = hi - lo
                sq = sqp_pool.tile([P, CHUNK], dt, tag="sq")
                nc.scalar.activation(sq[:, :fd], xc, AF.Square,
                                     bias=bias2_a, scale=s_a)
                t1 = wp.tile([P, CHUNK], dt, tag="t1")
                if ci % 2 == 1:
                    nc.scalar.activation(t1[:, :fd], xc, AF.Identity,
                                         bias=K2_d, scale=K1_a)
                else:
                    nc.vector.tensor_scalar(t1[:, :fd], xc, K1_a, K2_d,
                                            op0=OP.mult, op1=OP.add)
                nc.vector.scalar_tensor_tensor(xc, sq[:, :fd], float(D),
                                               t1[:, :fd],
                                               op0=OP.add, op1=OP.mult)
                nc.sync.dma_start(out=y_ext[t * P:(t + 1) * P, lo:hi], in_=xc)

    nc.compile()
    return nc


def kernel(x: np.ndarray, knots: np.ndarray, coeffs: np.ndarray) -> np.ndarray:
    from concourse.bass_utils import run_bass_kernel_spmd

    x = np.ascontiguousarray(np.asarray(x, dtype=np.float32))
    assert x.shape == (ROWS, COLS), x.shape

    a = _expand_cubic(knots, coeffs)
    r, U, D, a3 = _factor_cubic(a)

    key = (np.asarray(knots, np.float32).tobytes(),
           np.asarray(coeffs, np.float32).tobytes())
    if _CACHE.get("key") != key:
        _CACHE["nc"] = _build_program(r, U, D, a3)
        _CACHE["key"] = key
    nc = _CACHE["nc"]

    shards = [x[i * R_CORE:(i + 1) * R_CORE] for i in range(N_CORES)]
    in_maps = [{"x": s} for s in shards]

    import os
    trace = bool(int(os.environ.get("KERNEL_TRACE", "0")))
    res = run_bass_kernel_spmd(nc, in_maps, core_ids=list(range(N_CORES)),
                               trace=trace)
    if trace and res.exec_time_ns is not None:
        print(f"HW exec time: {res.exec_time_ns} ns")
        _CACHE["last_exec_time_ns"] = res.exec_time_ns
        _CACHE["last_trace"] = res.instructions_and_trace

    out = np.empty((ROWS, COLS), dtype=np.float32)
    for i in range(N_CORES):
        out[i * R_CORE:(i + 1) * R_CORE] = res.results[i]["y"]
    return out
